# revision 2
# baseline (speedup 1.0000x reference)
"""Trainium2 Bass kernel for nn_RNN_6167573037204 (v2 redesign).

LSTM (input=1, hidden=24, T=1024) over batch 2048 + tiny MLP head.
Data-parallel: batch sharded 8 ways (256 per core); weights replicated.

Design (see also _build_nc):
 - fp16 on-chip (PSUM accumulates fp32): fp16 matmuls run 1 cycle/col on
   the PE (fp32 = 4), fp16 SBUF DVE ops run in 2x mode.
 - Core-local batch n=256 split into halves A/B packed on PARTITION
   row-blocks (engine cost depends only on free-dim size).
 - ONE tanh per step covers all 4 gates: PSUM P [128, 2m]:
     X cols 0:m   rows 0:24 f_A | 32:56 f_B | 64:88 i_A | 96:120 i_B
     Y cols m:2m  rows 0:24 o_A | 32:56 o_B | 64:88 g_A | 96:120 g_B
   f,i,o rows carry a 0.5 scale in the weights (sigmoid-via-tanh with
   doubled states H=2h, C=2c folds every affine fixup into weights).
 - P = 2 MMX matmuls (K=4, rhs [x_A;1;x_B;1] streamed from HBM in
   64-step chunks) + 4 accumulating MMh matmuls (K=24, rhs ring h rows
   0:24 / 32:56).
 - RB=(tf+1)*C on Pool/gpsimd (reads cell PSUM without the DVE psum
   penalty) -> RAB rows 0:56;  RA=(ti+1)*tg on DVE (all-fp16 2x) ->
   RAB rows 64:120.
 - Cell combine on PE: psC' = 0.5*RB + 1.0*RA via two diagonal-lhsT
   matmuls, each with exactly ONE cross-engine wait.
 - TCt = tanh(0.5*psC') (ACT);  ring' = (to+1)*TCt (DVE).

All compute-engine APs start at partition 0/32/64/96; all 2-input
DVE/Pool ops have equal operand start partitions.  _strip_waits()
reduces every instruction to <=1 sync wait via engine-program-order
transitivity (matmult HW wait-slot limit).
"""

import numpy as np

H = 24
B = 2048
T = 1024
NCORES = 8
N = B // NCORES   # 256 batch per core
M_HALF = N // 2   # 128 cols per half
XCHUNK = 64       # steps of x staged per HBM DMA

_NC_CACHE = {}
_RB_ON_POOL = True


def _build_nc(t_steps=T, m=M_HALF, strip=True):
    import concourse.bass as bass
    import concourse.mybir as mybir
    import concourse.tile as tile
    from concourse.tile import add_dep_helper
    from contextlib import ExitStack

    f16 = mybir.dt.float16
    f32 = mybir.dt.float32
    AF = mybir.ActivationFunctionType
    ALU = mybir.AluOpType

    nc = bass.Bass()

    # const pack column layout (see _pack_consts)
    CW = 1523 + 3 * m
    d_cp = nc.declare_dram_parameter("const_pack", [128, CW], f16, isOutput=False)
    d_xq = nc.declare_dram_parameter("xq", [42, t_steps * m], f16, isOutput=False)
    d_out = nc.declare_dram_parameter("out", [1, 2 * m], f32, isOutput=True)

    nchunk = (t_steps + XCHUNK - 1) // XCHUNK

    with ExitStack() as ctx:
        tc = ctx.enter_context(tile.TileContext(nc))
        consts = ctx.enter_context(tc.tile_pool(name="consts", bufs=1))
        xq_pool = ctx.enter_context(tc.tile_pool(name="xqp", bufs=1))
        pP = ctx.enter_context(tc.tile_pool(name="pP", bufs=3, space="PSUM"))
        pC = ctx.enter_context(tc.tile_pool(name="pC", bufs=1, space="PSUM"))
        g_pool = ctx.enter_context(tc.tile_pool(name="gab", bufs=3))
        r_pool = ctx.enter_context(tc.tile_pool(name="rab", bufs=3))
        t_pool = ctx.enter_context(tc.tile_pool(name="tct", bufs=3))
        h_pool = ctx.enter_context(tc.tile_pool(name="ring", bufs=4))

        cp = consts.tile([128, CW], f16)
        nc.sync.dma_start(cp[:, :], d_cp[:, :])

        lhsT_hX = cp[0:98, 0:128]
        lhsT_hY = cp[0:98, 128:256]
        lhsT_cA = cp[0:120, 768:832]
        lhsT_cB = cp[0:56, 832:896]
        lhsT_w1A = cp[0:24, 936:992]
        lhsT_w1B = cp[0:56, 992:1048]
        lhsT_w2A = cp[0:24, 1048:1081]
        lhsT_w2B = cp[0:56, 1081:1114]
        lhsT_b1 = cp[0:1, 1114:1170]
        lhsT_b2 = cp[0:1, 1170:1203]
        lhsT_cG = cp[0:120, 1459:1523]
        C0 = cp[0:64, 1523:1523 + m]
        H0 = cp[0:56, 1523 + m:1523 + 2 * m]
        ones = cp[0:1, 1523 + 2 * m:1523 + 3 * m]

        # cell-state psum ping-pong (psC = C' = 2c')
        psC = [pC.tile([64, m], f32, tag=f"psC{k}", name=f"psC{k}")
               for k in range(2)]

        # ring generation tiles: 8 steps per gen, ping-pong. Per slot cols
        # s*m:(s+1)*m: rows 0:56 = doubled hidden (H-STT), rows 64:66 =
        # [x_A;1], 96:98 = [x_B;1] (one DMA per gen fills rows 64:98 for
        # all 8 slots). MMh rhs = rows 0:98 (K=98, zero lhsT rows where
        # unused) so x and bias ride the same matmul as h.
        RGEN = 8
        ngen = (t_steps + RGEN - 1) // RGEN
        rg = [xq_pool.tile([128, RGEN * m], f16, tag=f"rg{k}", name=f"rg{k}")
              for k in range(2)]
        fscratch = consts.tile([66, 4], f16)

        fences = []

        def emit_xdma(g):
            lo = g * RGEN * m
            hi = min((g + 1) * RGEN * m, t_steps * m)
            nc.sync.dma_start(rg[g % 2][56:98, 0:hi - lo], d_xq[0:42, lo:hi])
            # fence the DMA into the DVE stream so downstream MMh x-RAW
            # waits strip to their single H-wait; the fence must PRECEDE
            # the next DVE op in the final schedule (pinned via add_dep
            # below -- the scheduler would otherwise sink it).
            fences.append(nc.vector.tensor_copy(
                fscratch[64:66, 0:2], rg[g % 2][64:66, 0:2]))

        def pin_fences(next_dve_inst):
            while fences:
                add_dep_helper(next_dve_inst.ins, fences.pop().ins,
                               sync=True, reason="xdma fence order")

        emit_xdma(0)
        if ngen > 1:
            emit_xdma(1)

        # RDC(t): rows 0:56 = c(t-1) fp16 (Cs copy), rows 64:120 = RA(t).
        # Initial c0 copy FIRST on DVE (the ring-init copy MMh(0) waits on
        # then transitively implies it).
        rdc = r_pool.tile([120, m], f16, tag="RDC", name="rdc0")
        cs_init = nc.vector.tensor_copy(rdc[0:64, :], C0)
        pin_fences(cs_init)

        # initial hidden state (doubled) into ring gen-0 slot 0; pinned
        # after cs_init (and hence the fences) in the DVE stream
        ring_init = nc.vector.tensor_copy(rg[0][0:56, 0:m], H0)
        add_dep_helper(ring_init.ins, cs_init.ins, sync=True,
                       reason="init order")

        for t in range(t_steps):
            ring = rg[(t // RGEN) % 2][:, (t % RGEN) * m:(t % RGEN + 1) * m]
            Pt = pP.tile([128, 2 * m], f32, tag="P")
            nc.tensor.matmul(Pt[:, 0:m], lhsT_hX, ring[0:98, :],
                             start=True, stop=False)
            nc.tensor.matmul(Pt[:, m:2 * m], lhsT_hY, ring[0:98, :],
                             start=False, stop=True)
            if t % RGEN == 1 and t // RGEN + 2 < ngen:
                emit_xdma(t // RGEN + 2)

            GAB = g_pool.tile([120, 2 * m], f16, tag="GAB")
            nc.scalar.activation(GAB[:, :], Pt[0:120, :], AF.Tanh)

            # products as plain TT multiplies on DVE (fp16 2x mode); the
            # linear +c and +tg terms of the cell update ride the PE.
            # RA' = ti*tg -> RDC rows 64:120
            ra_i = nc.vector.tensor_tensor(
                rdc[64:120, :], GAB[64:120, 0:m], GAB[64:120, m:2 * m],
                op=ALU.mult,
            )
            pin_fences(ra_i)
            # RB = tf*c (c = rdc rows 0:56, written by Cs(t-1) on this same
            # engine -> pure program-order dep, no semaphore)
            RB = t_pool.tile([56, m], f16, tag="RB")
            nc.vector.tensor_tensor(
                RB[:, :], GAB[0:56, 0:m], rdc[0:56, :], op=ALU.mult,
            )
            # cell combine on PE: psC' = 2c' = tg + (c + ti*tg) + tf*c
            # MM_C3 rhs = GAB Y-cols (one ACT wait), MM_C1 rhs = RDC (one
            # DVE wait), MM_C2 rhs = RB (one DVE wait).
            pc_new = psC[t % 2]
            nc.tensor.matmul(pc_new[:, :], lhsT_cG, GAB[:, m:2 * m],
                             start=True, stop=False)
            nc.tensor.matmul(pc_new[:, :], lhsT_cA, rdc[0:120, :],
                             start=False, stop=False)
            nc.tensor.matmul(pc_new[:, :], lhsT_cB, RB[:, :],
                             start=False, stop=True)

            TCt = t_pool.tile([56, m], f16, tag="TCt")
            nc.scalar.activation(TCt[:, :], pc_new[0:56, :], AF.Tanh, scale=0.5)

            ring_next = rg[((t + 1) // RGEN) % 2][
                :, ((t + 1) % RGEN) * m:((t + 1) % RGEN + 1) * m]
            nc.vector.scalar_tensor_tensor(
                ring_next[0:56, :], GAB[0:56, m:2 * m], 1.0, TCt[:, :],
                op0=ALU.add, op1=ALU.mult,
            )
            # Cs(t) = 0.5*psC' = c(t), on DVE AFTER H and after ACT3 in
            # program order: reader-order dep lands on ACT3 (covered) and
            # next step's RB reads it through DVE program order alone.
            rdc_next = r_pool.tile([120, m], f16, tag="RDC")
            nc.vector.tensor_scalar_mul(rdc_next[0:64, :], pc_new[:, :], 0.5)

            rdc = rdc_next

        # ---- MLP head: z = relu(W1 h + b1); out = relu(W2 z + b2) ----
        ring = rg[(t_steps // RGEN) % 2][
            :, (t_steps % RGEN) * m:(t_steps % RGEN + 1) * m]
        psZ = pC.tile([56, m], f32, tag="psZ")
        nc.tensor.matmul(psZ[:, :], lhsT_b1, ones, start=True, stop=False)
        nc.tensor.matmul(psZ[:, :], lhsT_w1A, ring[0:24, :], start=False, stop=False)
        nc.tensor.matmul(psZ[:, :], lhsT_w1B, ring[0:56, :], start=False, stop=True)
        Z = t_pool.tile([56, m], f16, tag="Z")
        nc.scalar.activation(Z[:, :], psZ[:, :], AF.Relu)

        psO = pC.tile([33, m], f32, tag="psO")
        nc.tensor.matmul(psO[:, :], lhsT_b2, ones, start=True, stop=False)
        nc.tensor.matmul(psO[:, :], lhsT_w2A, Z[0:24, :], start=False, stop=False)
        nc.tensor.matmul(psO[:, :], lhsT_w2B, Z[0:56, :], start=False, stop=True)
        O = g_pool.tile([33, m], f32, tag="O")
        nc.scalar.activation(O[:, :], psO[:, :], AF.Relu)
        od1 = nc.sync.dma_start(d_out[0:1, 0:m], O[0:1, :])
        od2 = nc.sync.dma_start(d_out[0:1, m:2 * m], O[32:33, :])
        # drain fence: one single-wait DVE copy per out-DMA so the final
        # drain's DMA waits are transitively covered by its DVE wait
        fdummy = consts.tile([1, 8], f16)
        for k, od in enumerate((od1, od2)):
            cop = nc.vector.tensor_copy(fdummy[0:1, k:k + 1], cp[0:1, k:k + 1])
            add_dep_helper(cop.ins, od.ins, sync=True, reason="drain fence")

    if strip:
        _strip_waits(nc)
    return nc


def _strip_waits(nc):
    """Exact transitive reduction of sync waits (vector-clock based).

    Each engine increments its own semaphore(s) in program order and
    executes its stream in order.  For instruction i let clock[i][sem]
    be the highest count of `sem` known complete before i runs
    (propagated through same-engine program order and through kept
    waits).  A wait (sem >= v) is redundant if already covered; when
    several genuine waits remain we keep a minimal covering subset
    (preferring a single wait whose producer's clock implies the rest).
    This enforces the HW 1-wait slot on matmults without changing
    semantics: only implied edges are dropped.
    """
    insts = []
    for blk in nc.m.functions[0].blocks:
        insts.extend(blk.instructions)

    def stream_of(inst):
        return str(inst.engine)

    # per-sem cumulative counts along each engine stream; producer map
    cum = {}
    producer = {}          # (sem, value) -> inst index
    prev_on = {}           # stream -> last inst index
    clock = {}             # inst index -> dict sem -> count
    pruned = 0

    def join(a, b):
        for k, v in b.items():
            if a.get(k, 0) < v:
                a[k] = v

    for idx, inst in enumerate(insts):
        st = stream_of(inst)
        si = getattr(inst, "sync_info", None)
        c = dict(clock.get(prev_on.get(st, -1), {}))
        if si is not None and si.on_wait:
            waits = [(w, w.ant_name, w.wait_value) for w in si.on_wait]
            genuine = []
            keep_always = []
            for w, sem, val in waits:
                # only data-sem waits (engine-prefixed, >= semantics) are
                # eligible; barrier/gather sems use sub semantics and must
                # never be touched
                plain = (
                    str(w.wait_mode) == "sem-ge-imm"
                    and "barrier" not in sem and "gather" not in sem
                )
                if not plain:
                    keep_always.append((w, sem, val))
                    continue
                if c.get(sem, 0) >= val:
                    pruned += 1
                    continue
                genuine.append((w, sem, val))
            if len(genuine) > 1:
                # pick the single covering wait whose producer is EARLIEST
                # in the schedule (a later producer would stretch the chain)
                best, best_pc = None, None
                for w, sem, val in genuine:
                    pc = producer.get((sem, val))
                    cov = dict(c)
                    join(cov, clock.get(pc, {}))
                    cov[sem] = max(cov.get(sem, 0), val)
                    if all(cov.get(s2, 0) >= v2 for (_, s2, v2) in genuine):
                        if best is None or (pc or 0) < (best_pc or 0):
                            best, best_pc = (w, sem, val), pc
                if best is not None:
                    pruned += len(genuine) - 1
                    genuine = [best]
            for w, sem, val in genuine + keep_always:
                pc = producer.get((sem, val))
                join(c, clock.get(pc, {}))
                c[sem] = max(c.get(sem, 0), val)
            si.on_wait = [w for (w, _, _) in keep_always + genuine]
        # record own updates (visible after completion). DMA completion
        # sems fire asynchronously: register the producer (so waiters
        # inherit its issue-time clock) but do NOT credit the issuing
        # stream's clock with the increment.
        is_dma = type(inst).__name__ == "InstDMACopy"
        if si is not None and si.on_update:
            for u in si.on_update:
                sem = u.ant_name
                cum[sem] = cum.get(sem, 0) + (u.update_value or 1)
                producer[(sem, cum[sem])] = idx
                if not is_dma:
                    c[sem] = cum[sem]
        clock[idx] = c
        prev_on[st] = idx

    import collections
    left = collections.Counter(
        (type(i).__name__, len(i.sync_info.on_wait))
        for i in insts if getattr(i, "sync_info", None) and i.sync_info.on_wait
    )
    multi = {k: v for k, v in left.items() if k[1] > 1}
    assert not any(k[0] == "InstMatmult" for k in multi), multi
    return pruned, multi


def _pack_consts(h_state, c_state, W_ih, W_hh, b_ih, b_hh, W1, b1, W2, b2,
                 m=M_HALF):
    """Per-core fp16 const packs (list over cores)."""
    CW = 1523 + 3 * m
    Wh = np.asarray(W_hh, np.float64)
    Wi = np.asarray(W_ih, np.float64)
    b = np.asarray(b_ih, np.float64) + np.asarray(b_hh, np.float64)
    gi, gf, gg, go = slice(0, 24), slice(24, 48), slice(48, 72), slice(72, 96)

    base = np.zeros((128, CW), np.float32)

    def put_h(col0, gate, scale, block, row0=0):
        # lhsT_h*: [k, cols col0+block : +24] = Wh[gate]^T*scale; B variants
        # live at partition rows 32:56 (lhsT base must match rhs base).
        base[row0:row0 + 24, col0 + block:col0 + block + 24] = \
            Wh[gate, :].T * scale

    # lhsT_hX [98,128] at cols 0:128: out-cols 0:24 f_A | 32:56 f_B |
    # 64:88 i_A | 96:120 i_B; rows 0:24/32:56 = W_hh^T (A/B), rows 64:66 /
    # 96:98 = [W_ih; bias].  Scale 1/4 for f,i,o (0.5 sigmoid-via-tanh *
    # 1/2 doubled h); g uses 1/2 + x-scale 1.0.  hY at cols 128:256.
    def put_hx(col0, gate0, s0, gate1, s1):
        for blk, gate, sc in ((0, gate0, s0), (64, gate1, s1)):
            base[0:24, col0 + blk:col0 + blk + 24] = Wh[gate, :].T * sc / 2
            base[32:56, col0 + blk + 32:col0 + blk + 56] = Wh[gate, :].T * sc / 2
            base[64, col0 + blk:col0 + blk + 24] = Wi[gate, 0] * sc
            base[65, col0 + blk:col0 + blk + 24] = b[gate] * sc
            base[96, col0 + blk + 32:col0 + blk + 56] = Wi[gate, 0] * sc
            base[97, col0 + blk + 32:col0 + blk + 56] = b[gate] * sc

    put_hx(0, gf, 0.5, gi, 0.5)
    put_hx(128, go, 0.5, gg, 1.0)



    for c in list(range(24)) + list(range(32, 56)):
        base[c, 768 + c] = 1.0        # cA: psC += c      (RDC rows 0:56)
        base[64 + c, 768 + c] = 1.0   # cA: psC += ti*tg  (RDC rows 64:120)
        base[c, 832 + c] = 1.0        # cB: psC += tf*c
    for c in range(24):
        base[64 + c, 1459 + c] = 1.0       # cG: psC_A += tg_A
        base[96 + c, 1459 + 32 + c] = 1.0  # cG: psC_B += tg_B

    W1 = np.asarray(W1, np.float64)
    base[0:24, 936:960] = W1.T * 0.5           # w1A cols 0:24
    base[32:56, 1024:1048] = W1.T * 0.5        # w1B (partitions 32:56)
    base[0:24, 1048] = np.asarray(W2, np.float64)[0, :]    # w2A col 0
    base[32:56, 1113] = np.asarray(W2, np.float64)[0, :]   # w2B (parts 32:56)
    base[0, 1114:1138] = np.asarray(b1, np.float64)        # b1 A
    base[0, 1146:1170] = np.asarray(b1, np.float64)        # b1 B (1114+32)
    base[0, 1170] = float(np.asarray(b2)[0])               # b2 A
    base[0, 1202] = float(np.asarray(b2)[0])               # b2 B (1170+32)
    base[0, 1523 + 2 * m:1523 + 3 * m] = 1.0               # ones

    packs = []
    h_state = np.asarray(h_state, np.float32)
    c_state = np.asarray(c_state, np.float32)
    for core in range(NCORES):
        cpc = base.copy()
        lo = core * 2 * m
        cpc[0:24, 1523:1523 + m] = c_state[0, lo:lo + m, :].T
        cpc[32:56, 1523:1523 + m] = c_state[0, lo + m:lo + 2 * m, :].T
        cpc[0:24, 1523 + m:1523 + 2 * m] = 2.0 * h_state[0, lo:lo + m, :].T
        cpc[32:56, 1523 + m:1523 + 2 * m] = 2.0 * h_state[0, lo + m:lo + 2 * m, :].T
        packs.append(cpc.astype(np.float16))
    return packs


def _pack_xq(x, t_steps=T, m=M_HALF):
    """Per-core xq [42, t_steps*m] fp16 for the ring rows 56:98:
    rows 8:10 = [x_A; 1] (ring 64:66), rows 40:42 = [x_B; 1] (ring
    96:98), rest zero (covers the 56:64 gap so MMh reads are fully
    initialized)."""
    x = np.asarray(x, np.float32)
    packs = []
    for core in range(NCORES):
        lo = core * 2 * m
        xq = np.zeros((42, t_steps, m), np.float32)
        xq[8] = x[lo:lo + m, :t_steps, 0].T
        xq[9] = 1.0
        xq[40] = x[lo + m:lo + 2 * m, :t_steps, 0].T
        xq[41] = 1.0
        packs.append(np.ascontiguousarray(
            xq.reshape(42, t_steps * m)).astype(np.float16))
    return packs


def _prep_core_inputs(x, h_state, c_state, W_ih, W_hh, b_ih, b_hh,
                      W1, b1, W2, b2, t_steps=T, m=M_HALF):
    cps = _pack_consts(h_state, c_state, W_ih, W_hh, b_ih, b_hh,
                       W1, b1, W2, b2, m=m)
    xqs = _pack_xq(x, t_steps=t_steps, m=m)
    return [{"const_pack": cps[c], "xq": xqs[c]} for c in range(NCORES)]


def _run(in_maps, t_steps=T, m=M_HALF, trace=False, **kw):
    from concourse.bass_utils import run_bass_kernel_spmd

    key = (t_steps, m)
    if key not in _NC_CACHE:
        _NC_CACHE[key] = _build_nc(t_steps, m)
    nc = _NC_CACHE[key]
    return run_bass_kernel_spmd(nc, in_maps, list(range(NCORES)), trace=trace, **kw)


def kernel(x, h_state, c_state, y, W_ih, W_hh, b_ih, b_hh, W1, b1, W2, b2):
    in_maps = _prep_core_inputs(
        np.asarray(x), np.asarray(h_state), np.asarray(c_state),
        np.asarray(W_ih), np.asarray(W_hh), np.asarray(b_ih), np.asarray(b_hh),
        np.asarray(W1), np.asarray(b1), np.asarray(W2), np.asarray(b2),
    )
    res = _run(in_maps)
    out = np.concatenate([res.results[c]["out"][0] for c in range(NCORES)])
    return out.reshape(1, B, 1).astype(np.float32)
